# revision 17
# baseline (speedup 1.0000x reference)
"""Trainium2 Bass kernel v2 for nn_Attention (softmax(tanh(key @ (W @ query) + bias))).

Same contract as kernel.py. Differences, all knob-controlled:
  - DMA_QUEUES: alternate key-chunk DMAs between the two HWDGE rings
    (nc.sync / nc.scalar) instead of a single queue.
  - N_GPSIMD_PER_CHUNK: per 4-t-block chunk, this many t-blocks compute
    z on the GpSimd (tensor_tensor mult) + ScalarE (Copy+accum reduce)
    path instead of the DVE scalar_tensor_tensor path. GpSimd tiles
    accumulate into a separate z_g tile; merged with a DVE add before tanh.
  - USE_TTR: use DVE tensor_tensor_reduce instead of scalar_tensor_tensor.
"""

from contextlib import ExitStack

import numpy as np

import concourse.bacc as bacc
import concourse.mybir as mybir
import concourse.tile as tile
from concourse import masks
from concourse.bass_utils import run_bass_kernel_spmd

F32 = mybir.dt.float32
MULT = mybir.AluOpType.mult
ADD = mybir.AluOpType.add
AF = mybir.ActivationFunctionType

N_CORES = 8
B, T, Q, K = 64, 2048, 512, 512
B_LOC = B // N_CORES          # 8 batches per core
N_TBLK = T // 128             # 16 [128, K] tiles per batch
N_CHUNK = 8                   # DMA chunks per batch (512 KB each)
KEY_BUFS = 10                 # key tile pool depth (5 MB lookahead)
DMA_QUEUES = 2                # 1 = sync only, 2 = alternate sync/scalar
GP_MOD = 0                    # t-blocks with j%GP_MOD==GP_MOD-1 go GpSimd+ScalarE (0=off)
PIPELINE_EPILOGUE = True      # defer recip/norm/out of batch b into batch b+1
USE_TTR = False               # DVE op: tensor_tensor_reduce instead of STT
# "nmajor": t = (c*4+n)*128 + p -> 2KB HBM lines, out needs transpose
# "pmajor": t = p*16 + c*4 + n  -> 8KB HBM lines, out written directly
LAYOUT = "pmajor"
SCRATCH_PSUM = True           # STT main output goes to PSUM instead of SBUF
EPILOGUE_SCALAR = True        # rb copy + norm mul on ScalarE instead of DVE
# --- probe knobs (must be default for correctness) ---
STT_FAKE = False
STT_FIXED = False   # STT reads a const key-sized tile (DMA-untouched banks)
NO_STT = False
REPS = None
UNROLL = 8          # main_body copies per For_i iteration when REPS set


def emit(tc, ctx):
    nc = tc.nc
    query = nc.dram_tensor("query", [B_LOC, Q], F32, kind="ExternalInput").ap()
    key = nc.dram_tensor("key", [B_LOC, T, K], F32, kind="ExternalInput").ap()
    W = nc.dram_tensor("W", [K, Q], F32, kind="ExternalInput").ap()
    bias = nc.dram_tensor("bias", [1, 1], F32, kind="ExternalInput").ap()
    out = nc.dram_tensor("out", [B_LOC, T], F32, kind="ExternalOutput").ap()

    TBLK_PER_CHUNK = N_TBLK // N_CHUNK
    KC = K // 128  # 4 chunks of the k axis
    QC = Q // 128  # 4 chunks of the q axis

    const = ctx.enter_context(tc.tile_pool(name="const", bufs=1))
    key_pool = ctx.enter_context(tc.tile_pool(name="keyp", bufs=KEY_BUFS))
    gp_pool = ctx.enter_context(tc.tile_pool(name="gpp", bufs=3))
    z_pool = ctx.enter_context(tc.tile_pool(name="zp", bufs=2))
    ep_pool = ctx.enter_context(tc.tile_pool(name="epp", bufs=2))
    ps_setup = ctx.enter_context(tc.tile_pool(name="pss", bufs=2, space="PSUM"))
    ps_main = ctx.enter_context(tc.tile_pool(name="psm", bufs=3, space="PSUM"))
    ps_stt = ctx.enter_context(tc.tile_pool(name="psstt", bufs=2, space="PSUM"))

    # ---- constants ----
    identity = const.tile([128, 128], F32, tag="identity")
    masks.make_identity(nc, identity[:])
    ones_col = const.tile([128, 1], F32, tag="ones_col")
    nc.vector.memset(ones_col[:], 1.0)
    ones_row = const.tile([1, 128], F32, tag="ones_row")
    nc.vector.memset(ones_row[:], 1.0)
    sel = const.tile([B_LOC, B_LOC, 128], F32, tag="sel")
    nc.gpsimd.memset(sel[:], 0.0)
    nc.gpsimd.affine_select(
        out=sel[:],
        in_=sel[:],
        compare_op=mybir.AluOpType.not_equal,
        fill=1.0,
        base=0,
        pattern=[[-1, B_LOC], [0, 128]],
        channel_multiplier=1,
    )

    # ---- small inputs (ACT HWDGE queue; key uses the sync queue) ----
    W_sb = const.tile([128, KC, Q], F32, tag="W_sb")
    nc.scalar.dma_start(out=W_sb[:], in_=W.rearrange("(kc p) q -> p kc q", p=128))
    q_sb = const.tile([B_LOC, Q], F32, tag="q_sb")
    nc.scalar.dma_start(out=q_sb[:], in_=query)
    bias_sb = const.tile([1, 1], F32, tag="bias_sb")
    nc.scalar.dma_start(out=bias_sb[:], in_=bias)

    # ---- W^T via TensorE transposes: WT_sb[p, qc, k] = W[k, qc*128+p] ----
    WT_sb = const.tile([128, QC, K], F32, tag="WT_sb")
    for qc in range(QC):
        wt_ps = ps_setup.tile([128, K], F32, tag="s")
        for kc in range(KC):
            nc.tensor.transpose(
                wt_ps[:, kc * 128 : (kc + 1) * 128],
                W_sb[:, kc, qc * 128 : (qc + 1) * 128],
                identity[:],
            )
        nc.scalar.copy(WT_sb[:, qc, :], wt_ps[:])

    # ---- query^T: qT_sb[p, qc, b] = query[b, qc*128+p] ----
    qT_sb = const.tile([128, QC, B_LOC], F32, tag="qT_sb")
    for qc in range(QC):
        qt_ps = ps_setup.tile([128, B_LOC], F32, tag="s")
        nc.tensor.transpose(
            qt_ps[:],
            q_sb[:, qc * 128 : (qc + 1) * 128],
            identity[:B_LOC, :B_LOC],
        )
        nc.vector.tensor_copy(qT_sb[:, qc, :], qt_ps[:])

    # ---- mids[b, k] = sum_q W[k, q] query[b, q]  (true fp32 matmul) ----
    mids_ps = ps_setup.tile([B_LOC, K], F32, tag="s")
    for qc in range(QC):
        nc.tensor.matmul(
            mids_ps[:],
            qT_sb[:, qc, :],
            WT_sb[:, qc, :],
            start=(qc == 0),
            stop=(qc == QC - 1),
        )
    mids_sb = const.tile([B_LOC, K], F32, tag="mids_sb")
    nc.scalar.copy(mids_sb[:], mids_ps[:])

    # ---- bias broadcast to [128, 1] ----
    bb_ps = ps_setup.tile([128, 1], F32, tag="s")
    nc.tensor.matmul(bb_ps[:], ones_row[:], bias_sb[:], start=True, stop=True)
    bias_bc = const.tile([128, 1], F32, tag="bias_bc")
    nc.vector.tensor_copy(bias_bc[:], bb_ps[:])

    # ---- mids broadcast: mids_bc[p, b, k] = mids[b, k] for all p ----
    mids_bc = const.tile([128, B_LOC, K], F32, tag="mids_bc")
    for b in range(B_LOC):
        bc_ps = ps_setup.tile([128, K], F32, tag="s")
        nc.tensor.matmul(bc_ps[:], sel[:, b, :], mids_sb[:], start=True, stop=True)
        nc.scalar.copy(mids_bc[:, b, :], bc_ps[:])

    # ---- main loop ----
    scratch_v = const.tile([128, K], F32, tag="scratch_v")
    scratch_s = const.tile([128, K], F32, tag="scratch_s")
    out_sb = const.tile([N_TBLK, B_LOC, 128], F32, tag="out_sb")
    if STT_FIXED:
        kfix = const.tile([128, TBLK_PER_CHUNK, K], F32, tag="kfix")
        nc.vector.memset(kfix[:], 1.0)
    else:
        kfix = None

    if LAYOUT == "pmajor":
        key_r = key.rearrange("b (p c n) k -> b c p n k", n=TBLK_PER_CHUNK,
                              c=N_CHUNK)
        out_r = out.rearrange("b (p j) -> b p j", j=N_TBLK)
    else:
        key_r = key.rearrange("b (c n p) k -> b c p n k", n=TBLK_PER_CHUNK,
                              p=128)
        out_r = None

    def main_body():
        main_loop(tc, nc, key_pool, gp_pool, z_pool, ep_pool, ps_main, ps_stt,
                  key_r, out_r, mids_bc, bias_bc, scratch_v, scratch_s, out_sb,
                  identity, ones_col, ones_row, TBLK_PER_CHUNK, kfix)

    if REPS is None:
        main_body()
    else:
        u = UNROLL if (UNROLL and REPS % UNROLL == 0) else 1
        with tc.For_i(0, REPS // u, 1):
            for _ in range(u):
                main_body()

    if LAYOUT != "pmajor":
        nc.scalar.dma_start(
            out=out.rearrange("b (n p) -> n b p", p=128), in_=out_sb[:]
        )


def main_loop(tc, nc, key_pool, gp_pool, z_pool, ep_pool, ps_main, ps_stt,
              key_r, out_r, mids_bc, bias_bc, scratch_v, scratch_s, out_sb,
              identity, ones_col, ones_row, TBLK_PER_CHUNK, kfix=None):
    use_gp = GP_MOD > 0

    def epilogue_tail(st):
        # batch-b epilogue from the PSUM sum onwards; emitted late so the
        # DVE reciprocal never blocks the next batch's STT stream
        b, ex, sum_ps = st
        rec_sb = ep_pool.tile([1, 1], F32, tag="rec")
        nc.vector.reciprocal(rec_sb[:], sum_ps[:])
        rb_ps = ps_main.tile([128, 1], F32, tag="m")
        nc.tensor.matmul(rb_ps[:], ones_row[:], rec_sb[:], start=True, stop=True)
        rb_sb = ep_pool.tile([128, 1], F32, tag="rb")
        norm = ep_pool.tile([128, N_TBLK], F32, tag="norm")
        if EPILOGUE_SCALAR:
            nc.scalar.copy(rb_sb[:], rb_ps[:])
            nc.scalar.activation(norm[:], ex[:], AF.Copy, scale=rb_sb[:])
        else:
            nc.vector.tensor_copy(rb_sb[:], rb_ps[:])
            nc.vector.tensor_scalar_mul(norm[:], ex[:], rb_sb[:])
        if out_r is not None:
            # pmajor: norm[p, j] IS out[b, p*16+j]; DMA it out directly
            nc.scalar.dma_start(out=out_r[b], in_=norm[:])
        else:
            outT_ps = ps_main.tile([N_TBLK, 128], F32, tag="m")
            nc.tensor.transpose(outT_ps[:], norm[:], identity[:])
            nc.scalar.copy(out_sb[:, b, :], outT_ps[:])

    pending = None
    for b in range(B_LOC):
        z_t = z_pool.tile([128, N_TBLK], F32, tag="z")
        if use_gp:
            zg_t = z_pool.tile([128, N_TBLK], F32, tag="zg")
        else:
            zg_t = None
        if NO_STT:
            nc.vector.memset(z_t[:], 0.5)
        for c in range(N_CHUNK):
            key_t = key_pool.tile([128, TBLK_PER_CHUNK, K], F32, tag="key")
            dma_eng = nc.sync if (DMA_QUEUES == 1 or c % 2 == 0) else nc.scalar
            dma_eng.dma_start(out=key_t[:], in_=key_r[b, c])
            for n in range(TBLK_PER_CHUNK):
                j = c * TBLK_PER_CHUNK + n
                if NO_STT:
                    continue
                if STT_FAKE:
                    in0 = mids_bc[:, b, :]
                elif STT_FIXED:
                    in0 = kfix[:, n, :]
                else:
                    in0 = key_t[:, n, :]
                if use_gp and j % GP_MOD == GP_MOD - 1:
                    # GpSimd multiply + ScalarE fused copy-reduce
                    prod = gp_pool.tile([128, K], F32, tag="prod")
                    nc.gpsimd.tensor_tensor(
                        out=prod[:], in0=in0, in1=mids_bc[:, b, :], op=MULT
                    )
                    nc.scalar.activation(
                        scratch_s[:], prod[:], AF.Copy,
                        accum_out=zg_t[:, j : j + 1],
                    )
                elif USE_TTR:
                    if SCRATCH_PSUM:
                        ttr_out = ps_stt.tile([128, K], F32, tag="sps")
                        ttr_out = ttr_out[:]
                    else:
                        ttr_out = scratch_v[:]
                    nc.vector.tensor_tensor_reduce(
                        out=ttr_out,
                        in0=in0,
                        in1=mids_bc[:, b, :],
                        scale=1.0,
                        scalar=0.0,
                        op0=MULT,
                        op1=ADD,
                        accum_out=z_t[:, j : j + 1],
                    )
                else:
                    if SCRATCH_PSUM:
                        stt_out = ps_stt.tile([128, K], F32, tag="sps")
                        stt_out = stt_out[:]
                    else:
                        stt_out = scratch_v[:]
                    nc.vector.scalar_tensor_tensor(
                        out=stt_out,
                        in0=in0,
                        scalar=1.0,
                        in1=mids_bc[:, b, :],
                        op0=MULT,
                        op1=MULT,
                        accum_out=z_t[:, j : j + 1],
                    )
            if c == 1 and pending is not None:
                epilogue_tail(pending)
                pending = None

        if use_gp and not NO_STT:
            # the GpSimd path owns every GP_MOD-th z column; copy them
            # (disjoint from the DVE-written ones) into z_t
            zv = z_t[:].rearrange("p (c g) -> p c g", g=GP_MOD)[:, :, GP_MOD - 1 :]
            zg = zg_t[:].rearrange("p (c g) -> p c g", g=GP_MOD)[:, :, GP_MOD - 1 :]
            nc.vector.tensor_copy(zv, zg)

        th = ep_pool.tile([128, N_TBLK], F32, tag="th")
        nc.scalar.activation(th[:], z_t[:], AF.Tanh, bias=bias_bc[:], scale=1.0)
        ex = ep_pool.tile([128, N_TBLK], F32, tag="ex")
        exsum = ep_pool.tile([128, 1], F32, tag="exsum")
        nc.scalar.activation(ex[:], th[:], AF.Exp, accum_out=exsum[:])

        sum_ps = ps_main.tile([1, 1], F32, tag="m")
        nc.tensor.matmul(sum_ps[:], exsum[:], ones_col[:], start=True, stop=True)
        if PIPELINE_EPILOGUE:
            pending = (b, ex, sum_ps)
        else:
            epilogue_tail((b, ex, sum_ps))

    if pending is not None:
        epilogue_tail(pending)


_NC_CACHE = None


def build():
    global _NC_CACHE
    if _NC_CACHE is None:
        nc = bacc.Bacc(trn_type="TRN2", enable_partition_id=False)
        with tile.TileContext(nc) as tc:
            with ExitStack() as ctx:
                emit(tc, ctx)
        nc.compile()
        _NC_CACHE = nc
    return _NC_CACHE


def kernel(**inputs) -> np.ndarray:
    query = np.ascontiguousarray(np.asarray(inputs["query"], dtype=np.float32))
    key = np.ascontiguousarray(np.asarray(inputs["key"], dtype=np.float32))
    W = np.ascontiguousarray(np.asarray(inputs["W"], dtype=np.float32))
    bias = np.asarray(inputs["bias"], dtype=np.float32).reshape(1, 1)

    nc = build()
    in_maps = []
    for c in range(N_CORES):
        lo, hi = c * B_LOC, (c + 1) * B_LOC
        in_maps.append(
            {
                "query": np.ascontiguousarray(query[lo:hi]),
                "key": np.ascontiguousarray(key[lo:hi]),
                "W": W,
                "bias": bias,
            }
        )
    res = run_bass_kernel_spmd(nc, in_maps, core_ids=list(range(N_CORES)))
    return np.concatenate([res.results[c]["out"] for c in range(N_CORES)], axis=0)



# revision 19
# speedup vs baseline: 1.0236x; 1.0236x over previous
"""Trainium2 Bass kernel for nn_Attention (softmax(tanh(key @ (W @ query) + bias))).

Batch-sharded over 8 cores (8 batches each). Per batch: key streamed in
512 KiB chunks on the two HWDGE rings; DVE scalar_tensor_tensor computes
the key*mids products with free-dim accumulation into z; ScalarE does the
tanh/exp/normalize epilogue. Knobs:
  - DMA_QUEUES: alternate key-chunk DMAs between the two HWDGE rings.
  - PIPELINE_EPILOGUE: emit batch b's reciprocal/normalize/out-DMA one
    chunk into batch b+1 so the DVE never blocks on the tanh/exp chain.
  - EPILOGUE_SCALAR: rb copy + norm mul on ScalarE instead of DVE.
  - UNROLL: main_body copies per For_i iteration under REPS timing,
    amortizing the ~6us all-engine semaphore-reset barrier per iteration.
  - GP_MOD: GpSimd+ScalarE offload — leave 0; GpSimd shares SBUF ports
    with the DVE and running both slows STT ~2x.
  - USE_TTR: tensor_tensor_reduce — leave False; passes CoreSim but
    crashes on HW (NRT_EXEC_UNIT_UNRECOVERABLE).
"""

from contextlib import ExitStack

import numpy as np

import concourse.bacc as bacc
import concourse.mybir as mybir
import concourse.tile as tile
from concourse import masks
from concourse.bass_utils import run_bass_kernel_spmd

F32 = mybir.dt.float32
MULT = mybir.AluOpType.mult
ADD = mybir.AluOpType.add
AF = mybir.ActivationFunctionType

N_CORES = 8
B, T, Q, K = 64, 2048, 512, 512
B_LOC = B // N_CORES          # 8 batches per core
N_TBLK = T // 128             # 16 [128, K] tiles per batch
N_CHUNK = 8                   # DMA chunks per batch (512 KB each)
KEY_BUFS = 10                 # key tile pool depth (5 MB lookahead)
DMA_QUEUES = 2                # 1 = sync only, 2 = alternate sync/scalar
GP_MOD = 0                    # t-blocks with j%GP_MOD==GP_MOD-1 go GpSimd+ScalarE (0=off)
PIPELINE_EPILOGUE = True      # defer recip/norm/out of batch b into batch b+1
USE_TTR = False               # DVE op: tensor_tensor_reduce instead of STT
# "nmajor": t = (c*4+n)*128 + p -> 2KB HBM lines, out needs transpose
# "pmajor": t = p*16 + c*4 + n  -> 8KB HBM lines, out written directly
LAYOUT = "pmajor"
SCRATCH_PSUM = True           # STT main output goes to PSUM instead of SBUF
EPILOGUE_SCALAR = True        # rb copy + norm mul on ScalarE instead of DVE
# --- probe knobs (must be default for correctness) ---
STT_FAKE = False
STT_FIXED = False   # STT reads a const key-sized tile (DMA-untouched banks)
NO_STT = False
REPS = None
UNROLL = 4          # main_body copies per For_i iteration when REPS set


def emit(tc, ctx):
    nc = tc.nc
    query = nc.dram_tensor("query", [B_LOC, Q], F32, kind="ExternalInput").ap()
    key = nc.dram_tensor("key", [B_LOC, T, K], F32, kind="ExternalInput").ap()
    W = nc.dram_tensor("W", [K, Q], F32, kind="ExternalInput").ap()
    bias = nc.dram_tensor("bias", [1, 1], F32, kind="ExternalInput").ap()
    out = nc.dram_tensor("out", [B_LOC, T], F32, kind="ExternalOutput").ap()

    TBLK_PER_CHUNK = N_TBLK // N_CHUNK
    KC = K // 128  # 4 chunks of the k axis
    QC = Q // 128  # 4 chunks of the q axis

    const = ctx.enter_context(tc.tile_pool(name="const", bufs=1))
    key_pool = ctx.enter_context(tc.tile_pool(name="keyp", bufs=KEY_BUFS))
    gp_pool = ctx.enter_context(tc.tile_pool(name="gpp", bufs=3))
    z_pool = ctx.enter_context(tc.tile_pool(name="zp", bufs=2))
    ep_pool = ctx.enter_context(tc.tile_pool(name="epp", bufs=2))
    ps_setup = ctx.enter_context(tc.tile_pool(name="pss", bufs=2, space="PSUM"))
    ps_main = ctx.enter_context(tc.tile_pool(name="psm", bufs=3, space="PSUM"))
    ps_stt = ctx.enter_context(tc.tile_pool(name="psstt", bufs=2, space="PSUM"))

    # ---- constants ----
    identity = const.tile([128, 128], F32, tag="identity")
    masks.make_identity(nc, identity[:])
    ones_col = const.tile([128, 1], F32, tag="ones_col")
    nc.vector.memset(ones_col[:], 1.0)
    ones_row = const.tile([1, 128], F32, tag="ones_row")
    nc.vector.memset(ones_row[:], 1.0)
    sel = const.tile([B_LOC, B_LOC, 128], F32, tag="sel")
    nc.gpsimd.memset(sel[:], 0.0)
    nc.gpsimd.affine_select(
        out=sel[:],
        in_=sel[:],
        compare_op=mybir.AluOpType.not_equal,
        fill=1.0,
        base=0,
        pattern=[[-1, B_LOC], [0, 128]],
        channel_multiplier=1,
    )

    # ---- small inputs (ACT HWDGE queue; key uses the sync queue) ----
    W_sb = const.tile([128, KC, Q], F32, tag="W_sb")
    nc.scalar.dma_start(out=W_sb[:], in_=W.rearrange("(kc p) q -> p kc q", p=128))
    q_sb = const.tile([B_LOC, Q], F32, tag="q_sb")
    nc.scalar.dma_start(out=q_sb[:], in_=query)
    bias_sb = const.tile([1, 1], F32, tag="bias_sb")
    nc.scalar.dma_start(out=bias_sb[:], in_=bias)

    # ---- W^T via TensorE transposes: WT_sb[p, qc, k] = W[k, qc*128+p] ----
    WT_sb = const.tile([128, QC, K], F32, tag="WT_sb")
    for qc in range(QC):
        wt_ps = ps_setup.tile([128, K], F32, tag="s")
        for kc in range(KC):
            nc.tensor.transpose(
                wt_ps[:, kc * 128 : (kc + 1) * 128],
                W_sb[:, kc, qc * 128 : (qc + 1) * 128],
                identity[:],
            )
        nc.scalar.copy(WT_sb[:, qc, :], wt_ps[:])

    # ---- query^T: qT_sb[p, qc, b] = query[b, qc*128+p] ----
    qT_sb = const.tile([128, QC, B_LOC], F32, tag="qT_sb")
    for qc in range(QC):
        qt_ps = ps_setup.tile([128, B_LOC], F32, tag="s")
        nc.tensor.transpose(
            qt_ps[:],
            q_sb[:, qc * 128 : (qc + 1) * 128],
            identity[:B_LOC, :B_LOC],
        )
        nc.vector.tensor_copy(qT_sb[:, qc, :], qt_ps[:])

    # ---- mids[b, k] = sum_q W[k, q] query[b, q]  (true fp32 matmul) ----
    mids_ps = ps_setup.tile([B_LOC, K], F32, tag="s")
    for qc in range(QC):
        nc.tensor.matmul(
            mids_ps[:],
            qT_sb[:, qc, :],
            WT_sb[:, qc, :],
            start=(qc == 0),
            stop=(qc == QC - 1),
        )
    mids_sb = const.tile([B_LOC, K], F32, tag="mids_sb")
    nc.scalar.copy(mids_sb[:], mids_ps[:])

    # ---- bias broadcast to [128, 1] ----
    bb_ps = ps_setup.tile([128, 1], F32, tag="s")
    nc.tensor.matmul(bb_ps[:], ones_row[:], bias_sb[:], start=True, stop=True)
    bias_bc = const.tile([128, 1], F32, tag="bias_bc")
    nc.vector.tensor_copy(bias_bc[:], bb_ps[:])

    # ---- mids broadcast: mids_bc[p, b, k] = mids[b, k] for all p ----
    mids_bc = const.tile([128, B_LOC, K], F32, tag="mids_bc")
    for b in range(B_LOC):
        bc_ps = ps_setup.tile([128, K], F32, tag="s")
        nc.tensor.matmul(bc_ps[:], sel[:, b, :], mids_sb[:], start=True, stop=True)
        nc.scalar.copy(mids_bc[:, b, :], bc_ps[:])

    # ---- main loop ----
    scratch_v = const.tile([128, K], F32, tag="scratch_v")
    scratch_s = const.tile([128, K], F32, tag="scratch_s")
    out_sb = const.tile([N_TBLK, B_LOC, 128], F32, tag="out_sb")
    if STT_FIXED:
        kfix = const.tile([128, TBLK_PER_CHUNK, K], F32, tag="kfix")
        nc.vector.memset(kfix[:], 1.0)
    else:
        kfix = None

    if LAYOUT == "pmajor":
        key_r = key.rearrange("b (p c n) k -> b c p n k", n=TBLK_PER_CHUNK,
                              c=N_CHUNK)
        out_r = out.rearrange("b (p j) -> b p j", j=N_TBLK)
    else:
        key_r = key.rearrange("b (c n p) k -> b c p n k", n=TBLK_PER_CHUNK,
                              p=128)
        out_r = None

    def main_body():
        main_loop(tc, nc, key_pool, gp_pool, z_pool, ep_pool, ps_main, ps_stt,
                  key_r, out_r, mids_bc, bias_bc, scratch_v, scratch_s, out_sb,
                  identity, ones_col, ones_row, TBLK_PER_CHUNK, kfix)

    if REPS is None:
        main_body()
    else:
        u = UNROLL if (UNROLL and REPS % UNROLL == 0) else 1
        with tc.For_i(0, REPS // u, 1):
            for _ in range(u):
                main_body()

    if LAYOUT != "pmajor":
        nc.scalar.dma_start(
            out=out.rearrange("b (n p) -> n b p", p=128), in_=out_sb[:]
        )


def main_loop(tc, nc, key_pool, gp_pool, z_pool, ep_pool, ps_main, ps_stt,
              key_r, out_r, mids_bc, bias_bc, scratch_v, scratch_s, out_sb,
              identity, ones_col, ones_row, TBLK_PER_CHUNK, kfix=None):
    use_gp = GP_MOD > 0

    def epilogue_tail(st):
        # batch-b epilogue from the PSUM sum onwards; emitted late so the
        # DVE reciprocal never blocks the next batch's STT stream
        b, ex, sum_ps = st
        rec_sb = ep_pool.tile([1, 1], F32, tag="rec")
        nc.vector.reciprocal(rec_sb[:], sum_ps[:])
        rb_ps = ps_main.tile([128, 1], F32, tag="m")
        nc.tensor.matmul(rb_ps[:], ones_row[:], rec_sb[:], start=True, stop=True)
        rb_sb = ep_pool.tile([128, 1], F32, tag="rb")
        norm = ep_pool.tile([128, N_TBLK], F32, tag="norm")
        if EPILOGUE_SCALAR:
            nc.scalar.copy(rb_sb[:], rb_ps[:])
            nc.scalar.activation(norm[:], ex[:], AF.Copy, scale=rb_sb[:])
        else:
            nc.vector.tensor_copy(rb_sb[:], rb_ps[:])
            nc.vector.tensor_scalar_mul(norm[:], ex[:], rb_sb[:])
        if out_r is not None:
            # pmajor: norm[p, j] IS out[b, p*16+j]; DMA it out directly
            nc.scalar.dma_start(out=out_r[b], in_=norm[:])
        else:
            outT_ps = ps_main.tile([N_TBLK, 128], F32, tag="m")
            nc.tensor.transpose(outT_ps[:], norm[:], identity[:])
            nc.scalar.copy(out_sb[:, b, :], outT_ps[:])

    pending = None
    for b in range(B_LOC):
        z_t = z_pool.tile([128, N_TBLK], F32, tag="z")
        if use_gp:
            zg_t = z_pool.tile([128, N_TBLK], F32, tag="zg")
        else:
            zg_t = None
        if NO_STT:
            nc.vector.memset(z_t[:], 0.5)
        for c in range(N_CHUNK):
            key_t = key_pool.tile([128, TBLK_PER_CHUNK, K], F32, tag="key")
            dma_eng = nc.sync if (DMA_QUEUES == 1 or c % 2 == 0) else nc.scalar
            dma_eng.dma_start(out=key_t[:], in_=key_r[b, c])
            for n in range(TBLK_PER_CHUNK):
                j = c * TBLK_PER_CHUNK + n
                if NO_STT:
                    continue
                if STT_FAKE:
                    in0 = mids_bc[:, b, :]
                elif STT_FIXED:
                    in0 = kfix[:, n, :]
                else:
                    in0 = key_t[:, n, :]
                if use_gp and j % GP_MOD == GP_MOD - 1:
                    # GpSimd multiply + ScalarE fused copy-reduce
                    prod = gp_pool.tile([128, K], F32, tag="prod")
                    nc.gpsimd.tensor_tensor(
                        out=prod[:], in0=in0, in1=mids_bc[:, b, :], op=MULT
                    )
                    nc.scalar.activation(
                        scratch_s[:], prod[:], AF.Copy,
                        accum_out=zg_t[:, j : j + 1],
                    )
                elif USE_TTR:
                    if SCRATCH_PSUM:
                        ttr_out = ps_stt.tile([128, K], F32, tag="sps")
                        ttr_out = ttr_out[:]
                    else:
                        ttr_out = scratch_v[:]
                    nc.vector.tensor_tensor_reduce(
                        out=ttr_out,
                        in0=in0,
                        in1=mids_bc[:, b, :],
                        scale=1.0,
                        scalar=0.0,
                        op0=MULT,
                        op1=ADD,
                        accum_out=z_t[:, j : j + 1],
                    )
                else:
                    if SCRATCH_PSUM:
                        stt_out = ps_stt.tile([128, K], F32, tag="sps")
                        stt_out = stt_out[:]
                    else:
                        stt_out = scratch_v[:]
                    nc.vector.scalar_tensor_tensor(
                        out=stt_out,
                        in0=in0,
                        scalar=1.0,
                        in1=mids_bc[:, b, :],
                        op0=MULT,
                        op1=MULT,
                        accum_out=z_t[:, j : j + 1],
                    )
            if c == 1 and pending is not None:
                epilogue_tail(pending)
                pending = None

        if use_gp and not NO_STT:
            # the GpSimd path owns every GP_MOD-th z column; copy them
            # (disjoint from the DVE-written ones) into z_t
            zv = z_t[:].rearrange("p (c g) -> p c g", g=GP_MOD)[:, :, GP_MOD - 1 :]
            zg = zg_t[:].rearrange("p (c g) -> p c g", g=GP_MOD)[:, :, GP_MOD - 1 :]
            nc.vector.tensor_copy(zv, zg)

        th = ep_pool.tile([128, N_TBLK], F32, tag="th")
        nc.scalar.activation(th[:], z_t[:], AF.Tanh, bias=bias_bc[:], scale=1.0)
        ex = ep_pool.tile([128, N_TBLK], F32, tag="ex")
        exsum = ep_pool.tile([128, 1], F32, tag="exsum")
        nc.scalar.activation(ex[:], th[:], AF.Exp, accum_out=exsum[:])

        sum_ps = ps_main.tile([1, 1], F32, tag="m")
        nc.tensor.matmul(sum_ps[:], exsum[:], ones_col[:], start=True, stop=True)
        if PIPELINE_EPILOGUE:
            pending = (b, ex, sum_ps)
        else:
            epilogue_tail((b, ex, sum_ps))

    if pending is not None:
        epilogue_tail(pending)


_NC_CACHE = None


def build():
    global _NC_CACHE
    if _NC_CACHE is None:
        nc = bacc.Bacc(trn_type="TRN2", enable_partition_id=False)
        with tile.TileContext(nc) as tc:
            with ExitStack() as ctx:
                emit(tc, ctx)
        nc.compile()
        _NC_CACHE = nc
    return _NC_CACHE


def kernel(**inputs) -> np.ndarray:
    query = np.ascontiguousarray(np.asarray(inputs["query"], dtype=np.float32))
    key = np.ascontiguousarray(np.asarray(inputs["key"], dtype=np.float32))
    W = np.ascontiguousarray(np.asarray(inputs["W"], dtype=np.float32))
    bias = np.asarray(inputs["bias"], dtype=np.float32).reshape(1, 1)

    nc = build()
    in_maps = []
    for c in range(N_CORES):
        lo, hi = c * B_LOC, (c + 1) * B_LOC
        in_maps.append(
            {
                "query": np.ascontiguousarray(query[lo:hi]),
                "key": np.ascontiguousarray(key[lo:hi]),
                "W": W,
                "bias": bias,
            }
        )
    res = run_bass_kernel_spmd(nc, in_maps, core_ids=list(range(N_CORES)))
    return np.concatenate([res.results[c]["out"] for c in range(N_CORES)], axis=0)



# revision 25
# speedup vs baseline: 1.0297x; 1.0060x over previous
"""Trainium2 Bass kernel for nn_Attention (softmax(tanh(key @ (W @ query) + bias))).

Batch-sharded over 8 cores (8 batches each). Per batch: key streamed in
512 KiB chunks on the two HWDGE rings; DVE scalar_tensor_tensor computes
the key*mids products with free-dim accumulation into z; ScalarE does the
tanh/exp/normalize epilogue. Knobs:
  - DMA_QUEUES: alternate key-chunk DMAs between the two HWDGE rings.
  - PIPELINE_EPILOGUE: emit batch b's reciprocal/normalize/out-DMA one
    chunk into batch b+1 so the DVE never blocks on the tanh/exp chain.
  - EPILOGUE_SCALAR: rb copy + norm mul on ScalarE instead of DVE.
  - UNROLL: main_body copies per For_i iteration under REPS timing,
    amortizing the ~6us all-engine semaphore-reset barrier per iteration.
  - GP_MOD: GpSimd+ScalarE offload — leave 0; GpSimd shares SBUF ports
    with the DVE and running both slows STT ~2x.
  - USE_TTR: tensor_tensor_reduce — leave False; passes CoreSim but
    crashes on HW (NRT_EXEC_UNIT_UNRECOVERABLE).
"""

from contextlib import ExitStack

import numpy as np

import concourse.bacc as bacc
import concourse.mybir as mybir
import concourse.tile as tile
from concourse import masks
from concourse.bass_utils import run_bass_kernel_spmd

F32 = mybir.dt.float32
MULT = mybir.AluOpType.mult
ADD = mybir.AluOpType.add
AF = mybir.ActivationFunctionType

N_CORES = 8
B, T, Q, K = 64, 2048, 512, 512
B_LOC = B // N_CORES          # 8 batches per core
N_TBLK = T // 128             # 16 [128, K] tiles per batch
N_CHUNK = 8                   # DMA chunks per batch (512 KB each)
KEY_BUFS = 12                 # key tile pool depth (5 MB lookahead)
DMA_QUEUES = 2                # 1 = sync only, 2 = alternate sync/scalar
GP_MOD = 0                    # t-blocks with j%GP_MOD==GP_MOD-1 go GpSimd+ScalarE (0=off)
PIPELINE_EPILOGUE = True      # defer recip/norm/out of batch b into batch b+1
USE_TTR = False               # DVE op: tensor_tensor_reduce instead of STT
# "nmajor": t = (c*4+n)*128 + p -> 2KB HBM lines, out needs transpose
# "pmajor": t = p*16 + c*4 + n  -> 8KB HBM lines, out written directly
LAYOUT = "pmajor"
SCRATCH_PSUM = True           # STT main output goes to PSUM instead of SBUF
EPILOGUE_SCALAR = True        # rb copy + norm mul on ScalarE instead of DVE
# --- probe knobs (must be default for correctness) ---
STT_FAKE = False
STT_FIXED = False   # STT reads a const key-sized tile (DMA-untouched banks)
NO_STT = False
REPS = None
UNROLL = 4          # main_body copies per For_i iteration when REPS set


def emit(tc, ctx):
    nc = tc.nc
    query = nc.dram_tensor("query", [B_LOC, Q], F32, kind="ExternalInput").ap()
    key = nc.dram_tensor("key", [B_LOC, T, K], F32, kind="ExternalInput").ap()
    W = nc.dram_tensor("W", [K, Q], F32, kind="ExternalInput").ap()
    bias = nc.dram_tensor("bias", [1, 1], F32, kind="ExternalInput").ap()
    out = nc.dram_tensor("out", [B_LOC, T], F32, kind="ExternalOutput").ap()

    TBLK_PER_CHUNK = N_TBLK // N_CHUNK
    KC = K // 128  # 4 chunks of the k axis
    QC = Q // 128  # 4 chunks of the q axis

    const = ctx.enter_context(tc.tile_pool(name="const", bufs=1))
    key_pool = ctx.enter_context(tc.tile_pool(name="keyp", bufs=KEY_BUFS))
    gp_pool = ctx.enter_context(tc.tile_pool(name="gpp", bufs=3))
    z_pool = ctx.enter_context(tc.tile_pool(name="zp", bufs=2))
    ep_pool = ctx.enter_context(tc.tile_pool(name="epp", bufs=2))
    ps_setup = ctx.enter_context(tc.tile_pool(name="pss", bufs=2, space="PSUM"))
    ps_main = ctx.enter_context(tc.tile_pool(name="psm", bufs=3, space="PSUM"))
    ps_stt = ctx.enter_context(tc.tile_pool(name="psstt", bufs=2, space="PSUM"))

    # ---- constants ----
    identity = const.tile([128, 128], F32, tag="identity")
    masks.make_identity(nc, identity[:])
    ones_col = const.tile([128, 1], F32, tag="ones_col")
    nc.vector.memset(ones_col[:], 1.0)
    ones_row = const.tile([1, 128], F32, tag="ones_row")
    nc.vector.memset(ones_row[:], 1.0)
    sel = const.tile([B_LOC, B_LOC, 128], F32, tag="sel")
    nc.gpsimd.memset(sel[:], 0.0)
    nc.gpsimd.affine_select(
        out=sel[:],
        in_=sel[:],
        compare_op=mybir.AluOpType.not_equal,
        fill=1.0,
        base=0,
        pattern=[[-1, B_LOC], [0, 128]],
        channel_multiplier=1,
    )

    # ---- small inputs (ACT HWDGE queue; key uses the sync queue) ----
    W_sb = const.tile([128, KC, Q], F32, tag="W_sb")
    nc.scalar.dma_start(out=W_sb[:], in_=W.rearrange("(kc p) q -> p kc q", p=128))
    q_sb = const.tile([B_LOC, Q], F32, tag="q_sb")
    nc.scalar.dma_start(out=q_sb[:], in_=query)
    bias_sb = const.tile([1, 1], F32, tag="bias_sb")
    nc.scalar.dma_start(out=bias_sb[:], in_=bias)

    # ---- W^T via TensorE transposes: WT_sb[p, qc, k] = W[k, qc*128+p] ----
    WT_sb = const.tile([128, QC, K], F32, tag="WT_sb")
    for qc in range(QC):
        wt_ps = ps_setup.tile([128, K], F32, tag="s")
        for kc in range(KC):
            nc.tensor.transpose(
                wt_ps[:, kc * 128 : (kc + 1) * 128],
                W_sb[:, kc, qc * 128 : (qc + 1) * 128],
                identity[:],
            )
        nc.scalar.copy(WT_sb[:, qc, :], wt_ps[:])

    # ---- query^T: qT_sb[p, qc, b] = query[b, qc*128+p] ----
    qT_sb = const.tile([128, QC, B_LOC], F32, tag="qT_sb")
    for qc in range(QC):
        qt_ps = ps_setup.tile([128, B_LOC], F32, tag="s")
        nc.tensor.transpose(
            qt_ps[:],
            q_sb[:, qc * 128 : (qc + 1) * 128],
            identity[:B_LOC, :B_LOC],
        )
        nc.vector.tensor_copy(qT_sb[:, qc, :], qt_ps[:])

    # ---- mids[b, k] = sum_q W[k, q] query[b, q]  (true fp32 matmul) ----
    mids_ps = ps_setup.tile([B_LOC, K], F32, tag="s")
    for qc in range(QC):
        nc.tensor.matmul(
            mids_ps[:],
            qT_sb[:, qc, :],
            WT_sb[:, qc, :],
            start=(qc == 0),
            stop=(qc == QC - 1),
        )
    mids_sb = const.tile([B_LOC, K], F32, tag="mids_sb")
    nc.scalar.copy(mids_sb[:], mids_ps[:])

    # ---- bias broadcast to [128, 1] ----
    bb_ps = ps_setup.tile([128, 1], F32, tag="s")
    nc.tensor.matmul(bb_ps[:], ones_row[:], bias_sb[:], start=True, stop=True)
    bias_bc = const.tile([128, 1], F32, tag="bias_bc")
    nc.vector.tensor_copy(bias_bc[:], bb_ps[:])

    # ---- mids broadcast: mids_bc[p, b, k] = mids[b, k] for all p ----
    mids_bc = const.tile([128, B_LOC, K], F32, tag="mids_bc")
    for b in range(B_LOC):
        bc_ps = ps_setup.tile([128, K], F32, tag="s")
        nc.tensor.matmul(bc_ps[:], sel[:, b, :], mids_sb[:], start=True, stop=True)
        nc.scalar.copy(mids_bc[:, b, :], bc_ps[:])

    # ---- main loop ----
    scratch_v = const.tile([128, K], F32, tag="scratch_v")
    scratch_s = const.tile([128, K], F32, tag="scratch_s")
    out_sb = const.tile([N_TBLK, B_LOC, 128], F32, tag="out_sb")
    ob_pool = ctx.enter_context(tc.tile_pool(name="obp", bufs=2))
    if STT_FIXED:
        kfix = const.tile([128, TBLK_PER_CHUNK, K], F32, tag="kfix")
        nc.vector.memset(kfix[:], 1.0)
    else:
        kfix = None

    if LAYOUT == "pmajor":
        key_r = key.rearrange("b (p c n) k -> b c p n k", n=TBLK_PER_CHUNK,
                              c=N_CHUNK)
        # out[b, p*16+j] viewed as [p, b, j]: one 64 KB store per body
        # instead of 8 ring-disrupting 8 KB stores
        out_r = out.rearrange("b (p j) -> p b j", j=N_TBLK)
    else:
        key_r = key.rearrange("b (c n p) k -> b c p n k", n=TBLK_PER_CHUNK,
                              p=128)
        out_r = None

    def main_body():
        main_loop(tc, nc, key_pool, gp_pool, z_pool, ep_pool, ps_main, ps_stt,
                  key_r, out_r, mids_bc, bias_bc, scratch_v, scratch_s, out_sb,
                  ob_pool, identity, ones_col, ones_row, TBLK_PER_CHUNK, kfix)

    if REPS is None:
        main_body()
    else:
        u = UNROLL if (UNROLL and REPS % UNROLL == 0) else 1
        with tc.For_i(0, REPS // u, 1):
            for _ in range(u):
                main_body()

    if LAYOUT != "pmajor":
        nc.scalar.dma_start(
            out=out.rearrange("b (n p) -> n b p", p=128), in_=out_sb[:]
        )


def main_loop(tc, nc, key_pool, gp_pool, z_pool, ep_pool, ps_main, ps_stt,
              key_r, out_r, mids_bc, bias_bc, scratch_v, scratch_s, out_sb,
              ob_pool, identity, ones_col, ones_row, TBLK_PER_CHUNK, kfix=None):
    use_gp = GP_MOD > 0
    if out_r is not None:
        ob_t = ob_pool.tile([128, B_LOC, N_TBLK], F32, tag="ob")
    else:
        ob_t = None

    def epilogue_tail(st):
        # batch-b epilogue from the PSUM sum onwards; emitted late so the
        # DVE reciprocal never blocks the next batch's STT stream
        b, ex, sum_ps = st
        rec_sb = ep_pool.tile([1, 1], F32, tag="rec")
        nc.vector.reciprocal(rec_sb[:], sum_ps[:])
        rb_ps = ps_main.tile([128, 1], F32, tag="m")
        nc.tensor.matmul(rb_ps[:], ones_row[:], rec_sb[:], start=True, stop=True)
        rb_sb = ep_pool.tile([128, 1], F32, tag="rb")
        if out_r is not None:
            # pmajor: norm[p, j] IS out[b, p*16+j]; stage into ob_t
            norm = ob_t[:, b, :]
        else:
            norm_t = ep_pool.tile([128, N_TBLK], F32, tag="norm")
            norm = norm_t[:]
        if EPILOGUE_SCALAR:
            nc.scalar.copy(rb_sb[:], rb_ps[:])
            nc.scalar.activation(norm, ex[:], AF.Copy, scale=rb_sb[:])
        else:
            nc.vector.tensor_copy(rb_sb[:], rb_ps[:])
            nc.vector.tensor_scalar_mul(norm, ex[:], rb_sb[:])
        if out_r is None:
            outT_ps = ps_main.tile([N_TBLK, 128], F32, tag="m")
            nc.tensor.transpose(outT_ps[:], norm, identity[:])
            nc.scalar.copy(out_sb[:, b, :], outT_ps[:])

    pending = None
    for b in range(B_LOC):
        z_t = z_pool.tile([128, N_TBLK], F32, tag="z")
        if use_gp:
            zg_t = z_pool.tile([128, N_TBLK], F32, tag="zg")
        else:
            zg_t = None
        if NO_STT:
            nc.vector.memset(z_t[:], 0.5)
        for c in range(N_CHUNK):
            key_t = key_pool.tile([128, TBLK_PER_CHUNK, K], F32, tag="key")
            dma_eng = nc.sync if (DMA_QUEUES == 1 or c % 2 == 0) else nc.scalar
            dma_eng.dma_start(out=key_t[:], in_=key_r[b, c])
            for n in range(TBLK_PER_CHUNK):
                j = c * TBLK_PER_CHUNK + n
                if NO_STT:
                    continue
                if STT_FAKE:
                    in0 = mids_bc[:, b, :]
                elif STT_FIXED:
                    in0 = kfix[:, n, :]
                else:
                    in0 = key_t[:, n, :]
                if use_gp and j % GP_MOD == GP_MOD - 1:
                    # GpSimd multiply + ScalarE fused copy-reduce
                    prod = gp_pool.tile([128, K], F32, tag="prod")
                    nc.gpsimd.tensor_tensor(
                        out=prod[:], in0=in0, in1=mids_bc[:, b, :], op=MULT
                    )
                    nc.scalar.activation(
                        scratch_s[:], prod[:], AF.Copy,
                        accum_out=zg_t[:, j : j + 1],
                    )
                elif USE_TTR:
                    if SCRATCH_PSUM:
                        ttr_out = ps_stt.tile([128, K], F32, tag="sps")
                        ttr_out = ttr_out[:]
                    else:
                        ttr_out = scratch_v[:]
                    nc.vector.tensor_tensor_reduce(
                        out=ttr_out,
                        in0=in0,
                        in1=mids_bc[:, b, :],
                        scale=1.0,
                        scalar=0.0,
                        op0=MULT,
                        op1=ADD,
                        accum_out=z_t[:, j : j + 1],
                    )
                else:
                    if SCRATCH_PSUM:
                        stt_out = ps_stt.tile([128, K], F32, tag="sps")
                        stt_out = stt_out[:]
                    else:
                        stt_out = scratch_v[:]
                    nc.vector.scalar_tensor_tensor(
                        out=stt_out,
                        in0=in0,
                        scalar=1.0,
                        in1=mids_bc[:, b, :],
                        op0=MULT,
                        op1=MULT,
                        accum_out=z_t[:, j : j + 1],
                    )
            if c == 1 and pending is not None:
                epilogue_tail(pending)
                pending = None

        if use_gp and not NO_STT:
            # the GpSimd path owns every GP_MOD-th z column; copy them
            # (disjoint from the DVE-written ones) into z_t
            zv = z_t[:].rearrange("p (c g) -> p c g", g=GP_MOD)[:, :, GP_MOD - 1 :]
            zg = zg_t[:].rearrange("p (c g) -> p c g", g=GP_MOD)[:, :, GP_MOD - 1 :]
            nc.vector.tensor_copy(zv, zg)

        th = ep_pool.tile([128, N_TBLK], F32, tag="th")
        nc.scalar.activation(th[:], z_t[:], AF.Tanh, bias=bias_bc[:], scale=1.0)
        ex = ep_pool.tile([128, N_TBLK], F32, tag="ex")
        exsum = ep_pool.tile([128, 1], F32, tag="exsum")
        nc.scalar.activation(ex[:], th[:], AF.Exp, accum_out=exsum[:])

        sum_ps = ps_main.tile([1, 1], F32, tag="m")
        nc.tensor.matmul(sum_ps[:], exsum[:], ones_col[:], start=True, stop=True)
        if PIPELINE_EPILOGUE:
            pending = (b, ex, sum_ps)
        else:
            epilogue_tail((b, ex, sum_ps))

    if pending is not None:
        epilogue_tail(pending)
    if out_r is not None:
        nc.scalar.dma_start(out=out_r, in_=ob_t[:])


_NC_CACHE = None


def build():
    global _NC_CACHE
    if _NC_CACHE is None:
        nc = bacc.Bacc(trn_type="TRN2", enable_partition_id=False)
        with tile.TileContext(nc) as tc:
            with ExitStack() as ctx:
                emit(tc, ctx)
        nc.compile()
        _NC_CACHE = nc
    return _NC_CACHE


def kernel(**inputs) -> np.ndarray:
    query = np.ascontiguousarray(np.asarray(inputs["query"], dtype=np.float32))
    key = np.ascontiguousarray(np.asarray(inputs["key"], dtype=np.float32))
    W = np.ascontiguousarray(np.asarray(inputs["W"], dtype=np.float32))
    bias = np.asarray(inputs["bias"], dtype=np.float32).reshape(1, 1)

    nc = build()
    in_maps = []
    for c in range(N_CORES):
        lo, hi = c * B_LOC, (c + 1) * B_LOC
        in_maps.append(
            {
                "query": np.ascontiguousarray(query[lo:hi]),
                "key": np.ascontiguousarray(key[lo:hi]),
                "W": W,
                "bias": bias,
            }
        )
    res = run_bass_kernel_spmd(nc, in_maps, core_ids=list(range(N_CORES)))
    return np.concatenate([res.results[c]["out"] for c in range(N_CORES)], axis=0)

